# revision 2
# baseline (speedup 1.0000x reference)
"""Trainium2 Bass kernel for nn_InstDecoder (segment_reduce + bmm).

Computation (reference semantics):
  1. Per batch b: per-label masked mean of features over voxels
     inst[b, n, c] = mean_{v: labels[b,v]==n+1} features[b, c, v]   (labels 1..100)
  2. pred_kernel = inst @ Wk + bk                                   [B, 100, 64]
  3. pred_masks = pred_kernel @ mask_features.reshape(B, 64, M)     [B, 100, M]

Sharding: data-parallel over B (=2) x 4-way split of the flattened voxel axis
M = D*H*W = 524288 -> 8 cores, each owning a [*, 131072] voxel shard.

Phase 1 (device): per-core partial (sums, counts) over its shard via
one-hot(labels) matmuls accumulated in PSUM -> [65, 101] partials.
Host: sum partials across the 4 shards of each batch (26K adds, glue only).
Phase 2 (device): normalize by counts, apply Wk/bk, then the big bmm over the
mask_features shard -> [100, 131072] output shard per core.

Features are pre-transposed on the host during sharding (with a ones column
appended for the counts row) so the device kernels need no on-chip transpose
of the bulk data.
"""

import ml_dtypes
import numpy as np

BF16 = ml_dtypes.bfloat16

# ---- problem constants (hardcoded per contract) ----
B = 2
C = 64
KD = 64
D, H, W = 8, 256, 256
M = D * H * W            # 524288 voxels per batch
NUM_MASKS = 100
NL = NUM_MASKS + 1       # labels 0..100 (0 dropped at the end)
NSH = 4                  # voxel shards per batch
MSH = M // NSH           # 131072 voxels per core
NCORES = B * NSH

# phase-1 tiling: voxel subtiles of 128 (PE contraction dim), SUB per DMA chunk
P1_SUB = 32
P1_NCH = MSH // (128 * P1_SUB)   # 32 chunks of [128, SUB*65]

# phase-2 tiling: voxel chunks per DMA, matmul free dim 512 (one PSUM bank)
P2_CHUNK = 8192
P2_NCHU = MSH // P2_CHUNK        # 16
P2_NMM = P2_CHUNK // 512         # 16

_STATE = {}

# test.py can set this to a dict; per-phase HW exec times (ns) get stored.
PROFILE = None


def _tile_context(nc):
    """TileContext whose kernel-tail drain splits its semaphore waits into
    one wait_ge instruction each — this container's walrus rejects CTRL
    instructions carrying more than a couple of sync waits."""
    import concourse.tile as tile
    from concourse.vector_clock import ScopedClock

    class _SplitDrainTC(tile.TileContext):
        def _drain_and_barrier(self, tick_clock, wait_clock):
            nc = self.nc
            drain_inst = nc.sync.drain()
            wait_clock.add_sem_waits(
                drain_inst.ins, ScopedClock({None: tick_clock.global_clock}))
            si = drain_inst.ins.sync_info
            waits = list(si.on_wait) if si and si.on_wait else []
            handles = {s.name: s for s in self.sems.allocated().values()}
            if waits:
                si.on_wait = []
                for w in waits:
                    nc.sync.wait_ge(handles[w.ant_name], w.wait_value)
            nc.all_engine_barrier()
            popped = nc._tile_sem_poison_stack.pop()
            assert popped is self._sem_poison
            nc.clear_and_free_semaphores(list(self.sems.allocated().values()))
            nc.all_engine_barrier()

    return _SplitDrainTC(nc)


def _split_excess_waits(nc, max_waits=1):
    """This container's walrus rejects instructions carrying more than
    ~2 semaphore waits. Move excess waits onto same-engine nops inserted
    immediately before the offending instruction (monotonic sems make
    this semantically equivalent)."""
    import bass_rust

    created = {}
    new_names = set()
    for bb in nc.main_func.blocks:
        for ins in bb.instructions:
            if ins.name in new_names:
                continue
            si = ins.sync_info
            if si and si.on_wait and len(si.on_wait) > max_waits:
                waits = list(si.on_wait)
                si.on_wait = waits[:max_waits]
                extra = waits[max_waits:]
                nops = []
                for k in range(0, len(extra), max_waits):
                    n = nc.engines[ins.engine].nop(nofuse=True)
                    n.ins.sync_info = bass_rust.SyncInfo(
                        on_wait=extra[k:k + max_waits], on_update=[])
                    nops.append(n.ins)
                    new_names.add(n.ins.name)
                created[ins.name] = nops
    if not created:
        return
    for bb in nc.main_func.blocks:
        out = []
        for ins in bb.instructions:
            if ins.name in new_names:
                continue
            if ins.name in created:
                out.extend(created[ins.name])
            out.append(ins)
        bb.instructions = out


def _build_phase1():
    import concourse.bass as bass
    import concourse.mybir as mybir
    import concourse.tile as tile

    f32 = mybir.dt.float32
    bf16 = mybir.dt.bfloat16
    nc = bass.Bass()
    ft = nc.declare_dram_parameter("ft", [P1_NCH, 128, P1_SUB * 65], bf16, isOutput=False)
    lab = nc.declare_dram_parameter("lab", [P1_NCH, 128, P1_SUB], bf16, isOutput=False)
    iota = nc.declare_dram_parameter("iota", [128, NL], bf16, isOutput=False)
    part = nc.declare_dram_parameter("partials", [65, NL], f32, isOutput=True)

    with _tile_context(nc) as tc:
        with tc.tile_pool(name="const", bufs=1) as constp, \
             tc.tile_pool(name="io", bufs=3) as iop, \
             tc.tile_pool(name="oh", bufs=3) as ohp, \
             tc.tile_pool(name="ps", bufs=1, space="PSUM") as psp, \
             tc.tile_pool(name="out", bufs=1) as outp:
            iota_t = constp.tile([128, NL], bf16)
            nc.sync.dma_start(out=iota_t[:], in_=iota[:])
            acc = psp.tile([65, NL], f32)
            for c in range(P1_NCH):
                ftt = iop.tile([128, P1_SUB * 65], bf16, tag="ft")
                labt = iop.tile([128, P1_SUB], bf16, tag="lab")
                nc.sync.dma_start(out=ftt[:], in_=ft[c])
                nc.scalar.dma_start(out=labt[:], in_=lab[c])
                oht = ohp.tile([128, P1_SUB * NL], bf16)
                nc.vector.tensor_tensor(
                    out=oht[:].rearrange("p (s l) -> p s l", l=NL),
                    in0=labt[:, :, None].broadcast_to([128, P1_SUB, NL]),
                    in1=iota_t[:, None, :].broadcast_to([128, P1_SUB, NL]),
                    op=mybir.AluOpType.is_equal,
                )
                for j in range(P1_SUB):
                    nc.tensor.matmul(
                        acc[:],
                        lhsT=ftt[:, j * 65:(j + 1) * 65],
                        rhs=oht[:, j * NL:(j + 1) * NL],
                        start=(c == 0 and j == 0),
                        stop=(c == P1_NCH - 1 and j == P1_SUB - 1),
                    )
            out_t = outp.tile([65, NL], f32)
            nc.vector.tensor_copy(out=out_t[:], in_=acc[:])
            nc.sync.dma_start(out=part[:], in_=out_t[:])
    _split_excess_waits(nc)
    return nc


def _build_phase2():
    import concourse.bass as bass
    import concourse.mybir as mybir
    import concourse.tile as tile
    from concourse.masks import make_identity

    f32 = mybir.dt.float32
    bf16 = mybir.dt.bfloat16
    nc = bass.Bass()
    pt = nc.declare_dram_parameter("pt", [NL, 65], f32, isOutput=False)
    wk = nc.declare_dram_parameter("wk", [C, KD], f32, isOutput=False)
    bk = nc.declare_dram_parameter("bk", [KD, 1], f32, isOutput=False)
    mf = nc.declare_dram_parameter("mf", [C, MSH], bf16, isOutput=False)
    om = nc.declare_dram_parameter("om", [NUM_MASKS, MSH], bf16, isOutput=True)

    with _tile_context(nc) as tc:
        with tc.tile_pool(name="const", bufs=1) as constp, \
             tc.tile_pool(name="io", bufs=3) as iop, \
             tc.tile_pool(name="ob", bufs=2) as obp, \
             tc.tile_pool(name="ps", bufs=6, space="PSUM") as psp, \
             tc.tile_pool(name="ps2", bufs=1, space="PSUM") as psp2:
            pt_t = constp.tile([NL, 65], f32)
            nc.sync.dma_start(out=pt_t[:], in_=pt[:])
            wk_t = constp.tile([C, KD], f32)
            nc.sync.dma_start(out=wk_t[:], in_=wk[:])
            bk_t = constp.tile([KD, 1], f32)
            nc.sync.dma_start(out=bk_t[:], in_=bk[:])
            ident = constp.tile([NL, NL], f32)
            make_identity(nc, ident[:])

            # inst^T = (sums / max(counts, 1))^T, then PK^T = Wk^T @ inst^T + bk
            import concourse.mybir as mybir2
            cnt = constp.tile([NL, 1], f32)
            nc.vector.tensor_scalar(out=cnt[:], in0=pt_t[:, 64:65],
                                    scalar1=1.0, scalar2=None,
                                    op0=mybir2.AluOpType.max)
            rec = constp.tile([NL, 1], f32)
            nc.vector.reciprocal(out=rec[:], in_=cnt[:])
            snorm = constp.tile([NL, C], f32)
            nc.vector.tensor_scalar_mul(out=snorm[:], in0=pt_t[:, 0:C], scalar1=rec[:])
            instT_ps = psp2.tile([C, NL], f32)
            nc.tensor.transpose(out=instT_ps[:], in_=snorm[:], identity=ident[:])
            instT_sb = constp.tile([C, NL], f32)
            nc.vector.tensor_copy(out=instT_sb[:], in_=instT_ps[:])
            pkt_ps = psp2.tile([KD, NUM_MASKS], f32)
            nc.tensor.matmul(pkt_ps[:], lhsT=wk_t[:], rhs=instT_sb[:, 1:NL],
                             start=True, stop=True)
            pkt_sb = constp.tile([KD, NUM_MASKS], f32)
            nc.vector.tensor_scalar_add(out=pkt_sb[:], in0=pkt_ps[:], scalar1=bk_t[:, 0:1])
            pkt_bf = constp.tile([KD, NUM_MASKS], bf16)
            nc.vector.tensor_copy(out=pkt_bf[:], in_=pkt_sb[:])

            # big bmm: out[100, MSH] = PK^T.T @ mask_features
            # mf loads ride the sync HWDGE ring; om stores ride the scalar
            # ring so the two streams interleave at the SDMA engines.
            # Output buffers span 2 chunks to double the per-descriptor size.
            for ch in range(P2_NCHU):
                mft = iop.tile([C, P2_CHUNK], bf16, tag="mf")
                nc.gpsimd.dma_start(out=mft[:], in_=mf[:, ch * P2_CHUNK:(ch + 1) * P2_CHUNK])
                if ch % 2 == 0:
                    ob = obp.tile([NUM_MASKS, 2 * P2_CHUNK], bf16, tag="ob")
                half = (ch % 2) * P2_CHUNK
                for j in range(P2_NMM):
                    ps = psp.tile([NUM_MASKS, 512], f32)
                    nc.tensor.matmul(ps[:], lhsT=pkt_bf[:],
                                     rhs=mft[:, j * 512:(j + 1) * 512],
                                     start=True, stop=True)
                    if j % 2 == 0:
                        nc.vector.tensor_copy(out=ob[:, half + j * 512:half + (j + 1) * 512], in_=ps[:])
                    else:
                        nc.scalar.copy(out=ob[:, half + j * 512:half + (j + 1) * 512], in_=ps[:])
                if ch % 2 == 1:
                    eng = nc.scalar if (ch // 2) % 2 == 0 else nc.sync
                    eng.dma_start(
                        out=om[:, (ch - 1) * P2_CHUNK:(ch + 1) * P2_CHUNK], in_=ob[:])
    _split_excess_waits(nc)
    return nc


def _get_state():
    if not _STATE:
        _STATE["nc1"] = _build_phase1()
        _STATE["nc2"] = _build_phase2()
    return _STATE


def _run(nc, in_maps, tag):
    import os

    from concourse.bass_utils import run_bass_kernel_spmd

    trace = PROFILE is not None
    kw = {}
    tdir = os.environ.get("BASS_TRACE_DIR")
    if tdir:
        kw["tmpdir"] = os.path.join(tdir, tag)
        os.makedirs(kw["tmpdir"], exist_ok=True)
    res = run_bass_kernel_spmd(nc, in_maps, list(range(NCORES)), trace=trace, **kw)
    if PROFILE is not None:
        PROFILE[tag] = res.exec_time_ns
    return res.results


def kernel(features, mask_features, Wk, bk, init_masks):
    features = np.asarray(features, dtype=np.float32)
    mask_features = np.asarray(mask_features, dtype=np.float32)
    Wk = np.ascontiguousarray(np.asarray(Wk, dtype=np.float32))
    bk = np.asarray(bk, dtype=np.float32)
    init_masks = np.asarray(init_masks)

    st = _get_state()

    # ---- host-side sharding / layout prep ----
    feat = features.reshape(B, C, M)
    ftau = np.empty((B, M, 65), BF16)
    ftau[:, :, :C] = feat.transpose(0, 2, 1)
    ftau[:, :, C] = 1.0
    labf = init_masks.reshape(B, M).astype(BF16)
    iota = np.ascontiguousarray(
        np.broadcast_to(np.arange(NL, dtype=BF16)[None, :], (128, NL)))

    in_maps1 = []
    for b in range(B):
        for s in range(NSH):
            sl = slice(s * MSH, (s + 1) * MSH)
            in_maps1.append({
                "ft": ftau[b, sl].reshape(P1_NCH, 128, P1_SUB * 65),
                "lab": labf[b, sl].reshape(P1_NCH, 128, P1_SUB),
                "iota": iota,
            })
    r1 = _run(st["nc1"], in_maps1, "phase1")

    # combine shard partials per batch (tiny glue)
    parts = np.stack([r["partials"] for r in r1]).reshape(B, NSH, 65, NL).sum(axis=1)
    pts = np.ascontiguousarray(parts.transpose(0, 2, 1))  # [B, 101, 65]

    mfr = mask_features.reshape(B, C, M).astype(BF16)
    bk2 = np.ascontiguousarray(bk.reshape(KD, 1))
    in_maps2 = []
    for b in range(B):
        for s in range(NSH):
            sl = slice(s * MSH, (s + 1) * MSH)
            in_maps2.append({
                "pt": pts[b],
                "wk": Wk,
                "bk": bk2,
                "mf": mfr[b, :, sl],
            })
    r2 = _run(st["nc2"], in_maps2, "phase2")

    out = np.empty((B, NUM_MASKS, M), np.float32)
    for i in range(NCORES):
        b, s = divmod(i, NSH)
        out[b, :, s * MSH:(s + 1) * MSH] = r2[i]["om"]  # bf16 -> f32 upcast
    return out.reshape(B, NUM_MASKS, D, H, W)

